# revision 1
# baseline (speedup 1.0000x reference)
"""Cross-attention kernel for Trainium2, 8 NeuronCores SPMD.

Problem shapes (hardcoded): x [4,2048,1024], context [4,2048,1024],
mask [4,2048], HEADS=8, DIM_HEAD=64, INNER=512.

Sharding: core c handles batch b=c//2 and query-row half c%2 (1024 rows).
Each core computes all 8 heads over the full context for its rows; the
output is a disjoint [1024,1024] block -> gather is a pure concat.

Per-core dataflow (all matmul operands bf16, accumulation fp32 in PSUM):
  1. LN(x rows), LN(context) in natural layout, normalize -> bf16,
     PE-transpose 128x128 blocks -> xsT [dim,n], ctxT [dim,m] in SBUF.
  2. kT = (Wk' as lhsT).T @ ctxT   -> [inner, m]   (LN scale folded into W)
     V  = (ctxT as lhsT).T @ Wv'   -> [m, inner]   natural layout
     V_ext: per (m-chunk j, head h) slot of 65 cols = [V_h + bv | mask],
     rows scaled by mask -> masking and the softmax denominator both come
     for free out of the AV matmul.
  3. qT = (Wq' as lhsT).T @ xsT    -> [inner, n]   (q pre-scaled by d^-1/2)
  4. Attention per (head h, m-chunk j):
       simT[m128, n1024] = kT_hj.T-block @ qT_h   (PE, 2 matmuls N=512)
       pT = exp(simT)  (ACT, PSUM->SBUF bf16; no max-subtraction: logits
                        are ~N(0,1) after LN so exp cannot overflow)
       av[n128, 65] += pT-chunk.T @ V_ext_jh      (PE accumulation)
     plus null token: s0T[1,n] = k_null.T @ qT_h, e0 = exp(s0),
     av += e0-chunk.T @ [v_null | 1]  (rank-1, K=1 matmul, same PSUM group)
     Then r = 1/av[:,64] and attn_out[:, h*64:] = av[:, :64] * r.
  5. out = attn_outT @ Wo + bo -> DMA to DRAM.
"""

import numpy as np
import ml_dtypes

import concourse.bass as bass
import concourse.mybir as mybir
import concourse.tile as tile
from concourse import bacc
from concourse.masks import make_identity

F32 = mybir.dt.float32
BF16 = mybir.dt.bfloat16

P = 128
DIM = 1024
HEADS = 8
DH = 64
INNER = 512
N_CORE = 1024   # query rows per core
M = 2048        # context rows
NJ = M // P     # 16 context chunks
NQ = N_CORE // P  # 8 query chunks
KC = DIM // P   # 8 contraction chunks
EPS = 1e-6

_CACHE = {}


def build_program():
    nc = bacc.Bacc(None, target_bir_lowering=False)

    xs_d = nc.dram_tensor("xs", [N_CORE, DIM], F32, kind="ExternalInput")
    ctx_d = nc.dram_tensor("ctx", [M, DIM], F32, kind="ExternalInput")
    maskc_d = nc.dram_tensor("maskc", [P, NJ], F32, kind="ExternalInput")
    wq_d = nc.dram_tensor("wq", [DIM, INNER], BF16, kind="ExternalInput")
    wk_d = nc.dram_tensor("wk", [DIM, INNER], BF16, kind="ExternalInput")
    wv_d = nc.dram_tensor("wv", [DIM, INNER], BF16, kind="ExternalInput")
    wo_d = nc.dram_tensor("wo", [INNER, DIM], BF16, kind="ExternalInput")
    bq_d = nc.dram_tensor("bq", [1, INNER], BF16, kind="ExternalInput")
    bk_d = nc.dram_tensor("bk", [1, INNER], BF16, kind="ExternalInput")
    vb_d = nc.dram_tensor("vb", [1, INNER], BF16, kind="ExternalInput")
    bo_d = nc.dram_tensor("bo", [1, DIM], BF16, kind="ExternalInput")
    knull_d = nc.dram_tensor("knull", [P, 1], BF16, kind="ExternalInput")
    vne_d = nc.dram_tensor("vne", [1, 66], BF16, kind="ExternalInput")
    dencol_d = nc.dram_tensor("dencol", [P, NJ * HEADS], BF16, kind="ExternalInput")
    out_d = nc.dram_tensor("out", [N_CORE, DIM], F32, kind="ExternalOutput")

    with tile.TileContext(nc) as tc:
        with (
            tc.tile_pool(name="consts", bufs=1) as consts,
            tc.tile_pool(name="persist", bufs=1) as persist,
            tc.tile_pool(name="lnio", bufs=3) as lnio,
            tc.tile_pool(name="lnbf", bufs=3) as lnbf,
            tc.tile_pool(name="lntmp", bufs=4) as lntmp,
            tc.tile_pool(name="ptp", bufs=3) as ptp,
            tc.tile_pool(name="e0p", bufs=2) as e0p,
            tc.tile_pool(name="rp", bufs=2) as rp,
            tc.tile_pool(name="aop", bufs=2) as aop,
            tc.tile_pool(name="outp", bufs=2) as outp,
            tc.tile_pool(name="ps", bufs=2, space="PSUM") as psp,
            tc.tile_pool(name="av", bufs=2, space="PSUM") as avp,
        ):
            # ---- constants ----
            wq_sb = consts.tile([P, KC, INNER], BF16, tag="wq")
            nc.sync.dma_start(out=wq_sb, in_=wq_d.rearrange("(kc p) m -> p kc m", p=P))
            wk_sb = consts.tile([P, KC, INNER], BF16, tag="wk")
            nc.sync.dma_start(out=wk_sb, in_=wk_d.rearrange("(kc p) m -> p kc m", p=P))
            wv_sb = consts.tile([P, KC, INNER], BF16, tag="wv")
            nc.sync.dma_start(out=wv_sb, in_=wv_d.rearrange("(kc p) m -> p kc m", p=P))
            wo_sb = consts.tile([P, 4, DIM], BF16, tag="wo")
            nc.sync.dma_start(out=wo_sb, in_=wo_d.rearrange("(ic p) n -> p ic n", p=P))
            bqr_sb = consts.tile([1, INNER], BF16, tag="bqr")
            nc.sync.dma_start(out=bqr_sb, in_=bq_d[:])
            bkr_sb = consts.tile([1, INNER], BF16, tag="bkr")
            nc.sync.dma_start(out=bkr_sb, in_=bk_d[:])
            bvr_sb = consts.tile([1, INNER], BF16, tag="bvr")
            nc.sync.dma_start(out=bvr_sb, in_=vb_d[:])
            bor_sb = consts.tile([1, DIM], BF16, tag="bor")
            nc.sync.dma_start(out=bor_sb, in_=bo_d[:])
            ones_row = consts.tile([1, 512], BF16, tag="ones_row")
            nc.vector.memset(ones_row, 1.0)
            maskc_sb = consts.tile([P, NJ], F32, tag="maskc")
            nc.sync.dma_start(out=maskc_sb, in_=maskc_d[:])
            maskv_sb = consts.tile([P, NJ], F32, tag="maskv")
            nc.vector.tensor_copy(out=maskv_sb, in_=maskc_sb)
            knull_sb = consts.tile([P, 1], BF16, tag="knull")
            nc.sync.dma_start(out=knull_sb, in_=knull_d[:])
            vne_sb = consts.tile([1, 66], BF16, tag="vne")
            nc.sync.dma_start(out=vne_sb, in_=vne_d[:])
            ident = consts.tile([P, P], BF16, tag="ident")
            make_identity(nc, ident)
            eps_sb = consts.tile([P, 1], F32, tag="eps")
            nc.vector.memset(eps_sb, EPS)

            # ---- persistent activations ----
            ctxT = persist.tile([P, KC, M], BF16, tag="ctxT")
            xsT = persist.tile([P, KC, N_CORE], BF16, tag="xsT")
            kT = persist.tile([P, 4, M], BF16, tag="kT")
            vext = persist.tile([P, NJ, HEADS, 66], BF16, tag="vext")
            nc.sync.dma_start(out=vext[:, :, :, 64:65],
                              in_=dencol_d.rearrange("p (j h) -> p j h", j=NJ))
            qT = persist.tile([P, 4, N_CORE], BF16, tag="qT")
            attn_out = persist.tile([P, NQ, INNER], BF16, tag="attn_out")

            def ln_transpose(src_d, n_rows, dstT):
                for j in range(n_rows // P):
                    xt = lnio.tile([P, DIM], F32, tag="xt")
                    nc.sync.dma_start(out=xt, in_=src_d[j * P:(j + 1) * P, :])
                    stats = lntmp.tile([P, 2, 6], F32, tag="stats")
                    nc.vector.bn_stats(out=stats[:, 0, :], in_=xt[:, 0:512])
                    nc.vector.bn_stats(out=stats[:, 1, :], in_=xt[:, 512:1024])
                    mv = lntmp.tile([P, 2], F32, tag="mv")
                    nc.vector.bn_aggr(out=mv, in_=stats)
                    rstd = lntmp.tile([P, 1], F32, tag="rstd")
                    nc.scalar.activation(out=rstd, in_=mv[:, 1:2],
                                         func=mybir.ActivationFunctionType.Sqrt,
                                         bias=eps_sb)
                    nc.vector.reciprocal(out=rstd, in_=rstd)
                    xn = lnbf.tile([P, DIM], BF16, tag="xn")
                    nc.vector.tensor_scalar(
                        out=xn, in0=xt, scalar1=mv[:, 0:1], scalar2=rstd,
                        op0=mybir.AluOpType.subtract, op1=mybir.AluOpType.mult)
                    tp = psp.tile([P, KC * P], BF16, tag="ps")
                    for i in range(KC):
                        nc.tensor.transpose(out=tp[:, i * P:(i + 1) * P],
                                            in_=xn[:, i * P:(i + 1) * P],
                                            identity=ident)
                    for i in range(KC):
                        nc.scalar.copy(out=dstT[:, i, j * P:(j + 1) * P],
                                       in_=tp[:, i * P:(i + 1) * P])

            ln_transpose(ctx_d, M, ctxT)
            ln_transpose(xs_d, N_CORE, xsT)

            # ---- kT projection: [inner, m] ----
            for ic in range(4):
                for mh in range(4):
                    ps = psp.tile([P, 512], F32, tag="ps")
                    for kc in range(KC):
                        nc.tensor.matmul(
                            out=ps,
                            lhsT=wk_sb[:, kc, ic * P:(ic + 1) * P],
                            rhs=ctxT[:, kc, mh * 512:(mh + 1) * 512],
                            start=(kc == 0), stop=False)
                    nc.tensor.matmul(
                        out=ps, lhsT=bkr_sb[:, ic * P:(ic + 1) * P],
                        rhs=ones_row, start=False, stop=True)
                    nc.vector.tensor_copy(
                        out=kT[:, ic, mh * 512:(mh + 1) * 512], in_=ps)

            # ---- V projection (natural layout) + mask/bias -> V_ext ----
            for j in range(NJ):
                ps = avp.tile([P, 512], F32, tag="av")
                for kc in range(KC):
                    nc.tensor.matmul(
                        out=ps,
                        lhsT=ctxT[:, kc, j * P:(j + 1) * P],
                        rhs=wv_sb[:, kc, :],
                        start=(kc == 0), stop=False)
                nc.tensor.matmul(
                    out=ps, lhsT=ones_row[:, 0:P], rhs=bvr_sb,
                    start=False, stop=True)
                for h in range(HEADS):
                    nc.vector.tensor_scalar_mul(
                        out=vext[:, j, h, 0:64],
                        in0=ps[:, h * 64:(h + 1) * 64],
                        scalar1=maskv_sb[:, j:j + 1])

            # ---- q projection: [inner, n] ----
            for ic in range(4):
                for nh in range(2):
                    ps = psp.tile([P, 512], F32, tag="ps")
                    for kc in range(KC):
                        nc.tensor.matmul(
                            out=ps,
                            lhsT=wq_sb[:, kc, ic * P:(ic + 1) * P],
                            rhs=xsT[:, kc, nh * 512:(nh + 1) * 512],
                            start=(kc == 0), stop=False)
                    nc.tensor.matmul(
                        out=ps, lhsT=bqr_sb[:, ic * P:(ic + 1) * P],
                        rhs=ones_row, start=False, stop=True)
                    nc.vector.tensor_copy(
                        out=qT[:, ic, nh * 512:(nh + 1) * 512], in_=ps)

            # ---- attention ----
            for h in range(HEADS):
                hp = (h % 2) * DH
                ic = h // 2
                qh = qT[hp:hp + DH, ic, :]
                # null-token logits s0T[1, n] and e0 = exp(s0)
                s0 = psp.tile([1, N_CORE], F32, tag="ps")
                nc.tensor.matmul(out=s0[:, 0:512], lhsT=knull_sb[hp:hp + DH, :],
                                 rhs=qh[:, 0:512], start=True, stop=True)
                nc.tensor.matmul(out=s0[:, 512:1024], lhsT=knull_sb[hp:hp + DH, :],
                                 rhs=qh[:, 512:1024], start=True, stop=True)
                e0 = e0p.tile([1, N_CORE], BF16, tag="e0")
                nc.scalar.activation(out=e0, in_=s0,
                                     func=mybir.ActivationFunctionType.Exp)
                av = avp.tile([P, NQ, P], F32, tag="av")
                # PSUM start_tensor_calc zeroes a whole 2KB bank (4 of our
                # 128-f32 slots), so only the first matmul touching each bank
                # carries start=True; every slot's first write then lands on
                # still-pending-zero bytes and overwrites, later ones
                # accumulate. Group bookkeeping is bank-granular, hence
                # skip_group_check. The null-token rank-1 matmul opens each
                # slot (e0 is ready before the j loop).
                for q4 in range(NQ):
                    nc.tensor.matmul(
                        out=av[:, q4, 0:65],
                        lhsT=e0[:, q4 * P:(q4 + 1) * P],
                        rhs=vne_sb[:, 0:65],
                        start=(q4 % 4 == 0), stop=False,
                        skip_group_check=True)
                for j in range(NJ):
                    sm = psp.tile([P, N_CORE], F32, tag="ps")
                    kh = kT[hp:hp + DH, ic, j * P:(j + 1) * P]
                    nc.tensor.matmul(out=sm[:, 0:512], lhsT=kh, rhs=qh[:, 0:512],
                                     start=True, stop=True)
                    nc.tensor.matmul(out=sm[:, 512:1024], lhsT=kh,
                                     rhs=qh[:, 512:1024], start=True, stop=True)
                    pt = ptp.tile([P, N_CORE], BF16, tag="pt")
                    nc.scalar.activation(out=pt, in_=sm,
                                         func=mybir.ActivationFunctionType.Exp)
                    for q4 in range(NQ):
                        nc.tensor.matmul(
                            out=av[:, q4, 0:65],
                            lhsT=pt[:, q4 * P:(q4 + 1) * P],
                            rhs=vext[:, j, h, 0:65],
                            start=False, stop=(j == NJ - 1 and q4 % 4 == 3),
                            skip_group_check=True)
                r = rp.tile([P, NQ], F32, tag="r")
                for q4 in range(NQ):
                    nc.vector.reciprocal(out=r[:, q4:q4 + 1],
                                         in_=av[:, q4, 64:65])
                for q4 in range(NQ):
                    nc.vector.tensor_scalar_mul(
                        out=attn_out[:, q4, h * DH:(h + 1) * DH],
                        in0=av[:, q4, 0:64], scalar1=r[:, q4:q4 + 1])

            # ---- output projection ----
            for q4 in range(NQ):
                tp = psp.tile([P, 4 * P], BF16, tag="ps")
                for i in range(4):
                    nc.tensor.transpose(out=tp[:, i * P:(i + 1) * P],
                                        in_=attn_out[:, q4, i * P:(i + 1) * P],
                                        identity=ident)
                aoT = aop.tile([P, 4 * P], BF16, tag="aoT")
                nc.vector.tensor_copy(out=aoT, in_=tp)
                ot = outp.tile([P, DIM], F32, tag="ot")
                for oh in range(2):
                    ps = avp.tile([P, 512], F32, tag="av")
                    for ic in range(4):
                        nc.tensor.matmul(
                            out=ps, lhsT=aoT[:, ic * P:(ic + 1) * P],
                            rhs=wo_sb[:, ic, oh * 512:(oh + 1) * 512],
                            start=(ic == 0), stop=False)
                    nc.tensor.matmul(
                        out=ps, lhsT=ones_row[:, 0:P],
                        rhs=bor_sb[:, oh * 512:(oh + 1) * 512],
                        start=False, stop=True)
                    nc.vector.tensor_copy(
                        out=ot[:, oh * 512:(oh + 1) * 512], in_=ps)
                nc.sync.dma_start(out=out_d[q4 * P:(q4 + 1) * P, :], in_=ot)

    nc.compile()
    return nc


def prep_inputs(x, context, mask, ln_x_scale, ln_x_bias, ln_c_scale, ln_c_bias,
                Wq, bq, Wkv, bkv, Wo, bo, null_kv):
    """Host-side weight folding + per-core input maps."""
    f32 = np.float32
    bf16 = ml_dtypes.bfloat16
    scale = np.float32(DH ** (-0.5))
    x = np.asarray(x, f32)
    context = np.asarray(context, f32)
    mask = np.asarray(mask)
    Wq = np.asarray(Wq, f32)
    Wkv = np.asarray(Wkv, f32)
    Wo = np.asarray(Wo, f32)
    ln_x_scale = np.asarray(ln_x_scale, f32)
    ln_x_bias = np.asarray(ln_x_bias, f32)
    ln_c_scale = np.asarray(ln_c_scale, f32)
    ln_c_bias = np.asarray(ln_c_bias, f32)
    bq = np.asarray(bq, f32)
    bkv = np.asarray(bkv, f32)
    bo = np.asarray(bo, f32)
    null_kv = np.asarray(null_kv, f32)

    wq_f = (ln_x_scale[:, None] * Wq) * scale
    bq_f = (ln_x_bias @ Wq + bq) * scale
    wkv_f = ln_c_scale[:, None] * Wkv
    bkv_f = ln_c_bias @ Wkv + bkv
    wk_f, wv_f = wkv_f[:, :INNER], wkv_f[:, INNER:]
    bk_f, bv_f = bkv_f[:INNER], bkv_f[INNER:]

    shared = {
        "wq": np.ascontiguousarray(wq_f.astype(bf16)),
        "wk": np.ascontiguousarray(wk_f.astype(bf16)),
        "wv": np.ascontiguousarray(wv_f.astype(bf16)),
        "wo": np.ascontiguousarray(Wo.astype(bf16)),
        "bq": np.ascontiguousarray(bq_f.reshape(1, INNER).astype(bf16)),
        "bk": np.ascontiguousarray(bk_f.reshape(1, INNER).astype(bf16)),
        "vb": np.ascontiguousarray(bv_f.reshape(1, INNER).astype(bf16)),
        "bo": np.ascontiguousarray(bo.reshape(1, DIM).astype(bf16)),
        "knull": np.ascontiguousarray(
            np.tile(null_kv[0], 2).reshape(P, 1).astype(bf16)),
        "vne": np.ascontiguousarray(
            np.concatenate([null_kv[1], [1.0, 0.0]]).reshape(1, 66).astype(bf16)),
    }
    in_maps = []
    for c in range(8):
        b, half = c // 2, c % 2
        maskc = mask[b].astype(f32).reshape(NJ, P).T
        in_maps.append({
            "xs": np.ascontiguousarray(x[b, half * N_CORE:(half + 1) * N_CORE]),
            "ctx": np.ascontiguousarray(context[b]),
            "maskc": np.ascontiguousarray(maskc),
            "dencol": np.ascontiguousarray(np.repeat(maskc, HEADS, axis=1).astype(bf16)),
            **shared,
        })
    return in_maps


def kernel(**inputs):
    from concourse.bass_utils import run_bass_kernel_spmd

    if "nc" not in _CACHE:
        _CACHE["nc"] = build_program()
    nc = _CACHE["nc"]
    in_maps = prep_inputs(**inputs)
    res = run_bass_kernel_spmd(nc, in_maps, list(range(8)))
    out = np.empty((4, 2048, DIM), np.float32)
    for c in range(8):
        b, half = c // 2, c % 2
        out[b, half * N_CORE:(half + 1) * N_CORE] = res.results[c]["out"]
    return out

